# revision 25
# baseline (speedup 1.0000x reference)
"""Causal multi-head attention on 8 Trainium2 NeuronCores.

Problem: B=2, S=4096, D_MODEL=768, H=12, D_HEAD=64, fp32 I/O.

Sharding: (batch, head-group) -> core.  Cores 0-3 take batch 0, cores 4-7
take batch 1; each core computes 3 of the 12 heads for its batch and emits a
partial output [S, D_MODEL] (its heads' contribution to the W_O contraction).
The host sums the 4 partials per batch and adds b_O.

Per-core device program (matmul compute in bf16, fp32 PSUM accumulation):
  1. QT/KT[z, t] = W.T @ xT; heads 0,1 packed on partition halves (0-63 /
     64-127) so their scores matmuls run concurrently in different PE row
     groups; head 2 separate.  VT computed per head-pair/single, then
     PE-transposed to V[t, z] with a ones column appended (softmax row sums).
  2. Flash attention with scores in [k, q] layout so the exp output PT feeds
     the AV matmul directly; Z accumulates in PSUM [65 x W], row 64 = sum(P).
  3. Softmax normalization: row sums leave PSUM on DVE (32x32 block
     transposes + strided reciprocal), are broadcast across partitions by a
     rank-1 fp32r matmul, and applied with one DVE multiply.  The broadcast
     matmul for window w is emitted after window w+1's score matmuls so its
     DVE-side inputs are always ready and the PE never stalls (stalling >3.4us
     re-throttles the PE clock from 2.4 to 1.2 GHz).
  4. Output projection accumulates all 3 heads into PSUM [t, 768]; emitted
     interleaved with the head-2 windows to keep the PE dense.
"""

import numpy as np
import ml_dtypes

B, S, DM, H, DH = 2, 4096, 768, 12, 64
NCORES = 8
GROUPS = 4                  # head-groups per batch
HPC = H // GROUPS           # heads per core = 3
P = 128
QCH = 512                   # psum bank width (fp32)

_BF = ml_dtypes.bfloat16

_cache = {}


def _build(seq_len, use_biases):
    import concourse.bacc as bacc
    import concourse.mybir as mybir
    import concourse.tile as tile

    f32 = mybir.dt.float32
    f32r = mybir.dt.float32r
    bf16 = mybir.dt.bfloat16
    Exp = mybir.ActivationFunctionType.Exp
    mult = mybir.AluOpType.mult

    SQ = seq_len
    n_kt = SQ // P               # k tiles
    n_tt = SQ // P               # output row tiles
    n_ch = SQ // QCH             # 512-wide chunks
    DSL = DM // P                # contraction slices for the projections
    QS2 = min(2 * QCH, SQ)       # head-2 flash window
    n_w2 = SQ // QS2
    kpw2 = QS2 // P

    nc = bacc.Bacc(None, target_bir_lowering=False)

    xT = nc.declare_dram_parameter("xT", [DM, SQ], bf16, isOutput=False)
    wq = nc.declare_dram_parameter("wq", [DM, HPC * DH], bf16, isOutput=False)
    wk = nc.declare_dram_parameter("wk", [DM, HPC * DH], bf16, isOutput=False)
    wv = nc.declare_dram_parameter("wv", [DM, HPC * DH], bf16, isOutput=False)
    wo = nc.declare_dram_parameter("wo", [DH, HPC * DM], bf16, isOutput=False)
    trimask = nc.declare_dram_parameter("trimask", [P, P], bf16, isOutput=False)
    ident_b = nc.declare_dram_parameter("ident_b", [P, P], bf16, isOutput=False)
    ones_z = nc.declare_dram_parameter("ones_z", [1, DH], f32r, isOutput=False)
    if use_biases:
        bqkv_p = nc.declare_dram_parameter("bqkv_p", [P, 3], f32, isOutput=False)
        bqkv_s = nc.declare_dram_parameter("bqkv_s", [DH, 3], f32, isOutput=False)
    out = nc.declare_dram_parameter("out", [SQ, DM], f32, isOutput=True)

    with tile.TileContext(nc) as tc:
        with (
            tc.tile_pool(name="singles", bufs=1) as singles,
            tc.tile_pool(name="persist", bufs=1) as persist,
            tc.tile_pool(name="nrm_t", bufs=2) as nrm_t,
            tc.tile_pool(name="nrm_k", bufs=4) as nrm_k,
        ):
            # ---- constants / weights ----
            w_sb = {}
            for name, drm in (("q", wq), ("k", wk), ("v", wv)):
                t = singles.tile([P, DSL, HPC * DH], bf16, tag=f"w{name}")
                nc.sync.dma_start(t[:], drm.rearrange("(o p) c -> p o c", p=P))
                w_sb[name] = t
            wo_sb = singles.tile([DH, HPC, DM], bf16)
            nc.sync.dma_start(wo_sb[:], wo.rearrange("z (h d) -> z h d", h=HPC))
            tri_sb = singles.tile([P, P], bf16)
            nc.sync.dma_start(tri_sb[:], trimask[:])
            idb_sb = singles.tile([P, P], bf16)
            nc.sync.dma_start(idb_sb[:], ident_b[:])
            ones_sb = singles.tile([1, DH], f32r)
            nc.sync.dma_start(ones_sb[:], ones_z[:])
            bias_p = bias_s = None
            if use_biases:
                bias_p = singles.tile([P, 3], f32, tag="bp")
                nc.sync.dma_start(bias_p[:], bqkv_p[:])
                bias_s = singles.tile([DH, 3], f32, tag="bs")
                nc.sync.dma_start(bias_s[:], bqkv_s[:])

            # ---- persistent activations ----
            QT2 = persist.tile([P, SQ], bf16, tag="QT2")   # heads 0,1 stacked
            KT2 = persist.tile([P, SQ], bf16, tag="KT2")
            QTs = persist.tile([DH, SQ], bf16, tag="QTs")  # head 2
            KTs = persist.tile([DH, SQ], bf16, tag="KTs")
            V_sb = persist.tile([P, HPC, n_kt, DH + 1], bf16, tag="V")
            Zn_sb = persist.tile([DH, HPC, SQ], bf16, tag="Zn")

            # ================= QKV projections =================
            with (
                tc.tile_pool(name="xT_pool", bufs=1) as xT_pool,
                tc.tile_pool(name="qkv_ps", bufs=3, space="PSUM") as qkv_ps,
                tc.tile_pool(name="vt_ps", bufs=3, space="PSUM") as vt_ps,
                tc.tile_pool(name="vt_sb", bufs=1) as vt_pool,
            ):
                xT_sb = xT_pool.tile([P, DSL, SQ], bf16)
                for c in range(n_ch):
                    for o in range(DSL):
                        nc.sync.dma_start(
                            xT_sb[:, o, c * QCH:(c + 1) * QCH],
                            xT[o * P:(o + 1) * P, c * QCH:(c + 1) * QCH])

                def proj(tname, w_cols, dst, bias, c):
                    m = w_cols.stop - w_cols.start
                    ps = qkv_ps.tile([P, QCH], f32, tag="proj",
                                     name="proj_ps")[:m]
                    for o in range(DSL):
                        nc.tensor.matmul(
                            ps[:],
                            lhsT=w_sb[tname][:, o, w_cols],
                            rhs=xT_sb[:, o, c * QCH:(c + 1) * QCH],
                            start=(o == 0), stop=(o == DSL - 1),
                        )
                    if bias is not None:
                        nc.scalar.add(dst, ps[:], bias)
                    else:
                        nc.vector.tensor_copy(dst, ps[:])

                for tname, d2, ds, bi in (("q", QT2, QTs, 0),
                                          ("k", KT2, KTs, 1)):
                    for c in range(n_ch):
                        proj(tname, slice(0, P),
                             d2[:, c * QCH:(c + 1) * QCH],
                             bias_p[:, bi:bi + 1] if use_biases else None, c)
                    for c in range(n_ch):
                        proj(tname, slice(P, P + DH),
                             ds[:, c * QCH:(c + 1) * QCH],
                             bias_s[:, bi:bi + 1] if use_biases else None, c)
                # V: pair pass (M=128) + single pass, then batched transposes
                vt2 = vt_pool.tile([P, SQ], bf16, tag="vt2")
                for c in range(n_ch):
                    proj("v", slice(0, P), vt2[:, c * QCH:(c + 1) * QCH],
                         bias_p[:, 2:3] if use_biases else None, c)
                vts = vt_pool.tile([DH, SQ], bf16, tag="vts")
                for c in range(n_ch):
                    proj("v", slice(P, P + DH),
                         vts[:, c * QCH:(c + 1) * QCH],
                         bias_s[:, 2:3] if use_biases else None, c)
                for kt in range(n_kt):
                    for h, src, idsl in (
                            (0, vt2[0:DH, kt * P:(kt + 1) * P],
                             idb_sb[:DH, :DH]),
                            (1, vt2[DH:P, kt * P:(kt + 1) * P],
                             idb_sb[DH:P, DH:P]),
                            (2, vts[:, kt * P:(kt + 1) * P],
                             idb_sb[:DH, :DH])):
                        vp = vt_ps.tile([P, DH], bf16, tag="vtp", name="vp")
                        nc.tensor.transpose(vp[:], src, idsl)
                        nc.vector.tensor_copy(V_sb[:, h, kt, 0:DH], vp[:])
                nc.vector.memset(V_sb[:, :, :, DH:DH + 1], 1.0)

            # ---- normalization helpers (DVE stage + deferred PE stage) ----
            def norm_stage1(zacc, width):
                """All-DVE: extract row sums, reciprocal, repack; copy Z."""
                rr_pieces = []
                for j in range(width // QCH):
                    sl = slice(j * QCH, (j + 1) * QCH)
                    r32 = nrm_t.tile([32, QCH], f32, tag="r32", name="r32")
                    nc.vector.tensor_copy(r32[0:1, :], zacc[DH:DH + 1, sl])
                    rT = nrm_t.tile([32, QCH], f32, tag="rT", name="rT")
                    nc.vector.transpose(rT[:], r32[:])
                    rrT = nrm_t.tile([32, QCH], f32, tag="rrT", name="rrT")
                    nc.vector.reciprocal(
                        rrT.rearrange("p (j c) -> p j c", c=32)[:, :, 0],
                        rT.rearrange("p (j c) -> p j c", c=32)[:, :, 0])
                    rr32 = nrm_t.tile([32, QCH], f32, tag="rr32", name="rr32")
                    nc.vector.transpose(rr32[:], rrT[:])
                    rr_sb = nrm_k.tile([1, QCH], f32r, tag="rr", name="rr_sb")
                    nc.vector.tensor_copy(rr_sb[:], rr32[0:1, :])
                    rr_pieces.append(rr_sb)
                zsb = nrm_k.tile([DH, width], bf16, tag="zsb", name="zsb")
                nc.vector.tensor_copy(zsb[:], zacc[0:DH, :])
                return rr_pieces, zsb

            def norm_stage2(ps_pool, ps_tag, ps_shape, h, q0, width, staged):
                """PE rank-1 broadcast of 1/r, then one DVE multiply."""
                rr_pieces, zsb = staged
                rrb = ps_pool.tile(ps_shape, f32, tag=ps_tag, name="rrb")
                for half in range(width // QCH):
                    sl = slice(half * QCH, (half + 1) * QCH)
                    nc.tensor.matmul(rrb[:DH, sl], lhsT=ones_sb[:],
                                     rhs=rr_pieces[half][:],
                                     start=True, stop=True)
                nc.vector.tensor_tensor(
                    Zn_sb[:, h, q0:q0 + width], zsb[:], rrb[:DH, :width], mult)

            # ================= flash: head pair (0,1) =================
            with (
                tc.tile_pool(name="s_ps", bufs=2, space="PSUM") as s_ps,
                tc.tile_pool(name="z_ps", bufs=4, space="PSUM") as z_ps,
                tc.tile_pool(name="pt_sb", bufs=3) as pt_pool,
            ):
                staged = {}
                for qs in range(n_ch):
                    q0 = qs * QCH
                    za = z_ps.tile([DH + 1, QCH], f32, tag="zacc", name="za")
                    zb = z_ps.tile([DH + 1, QCH], f32, tag="zacc", name="zb")
                    nk = 4 * qs + 4
                    for ki in range(nk):
                        vs = max(0, P * ki - q0)
                        ssc = s_ps.tile([P, 2 * QCH], f32, tag="S", name="ssc")
                        nc.tensor.matmul(
                            ssc[:, vs:QCH],
                            lhsT=KT2[0:DH, ki * P:(ki + 1) * P],
                            rhs=QT2[0:DH, q0 + vs:q0 + QCH],
                            start=True, stop=True)
                        nc.tensor.matmul(
                            ssc[:, QCH + vs:2 * QCH],
                            lhsT=KT2[DH:P, ki * P:(ki + 1) * P],
                            rhs=QT2[DH:P, q0 + vs:q0 + QCH],
                            start=True, stop=True)
                        pt = pt_pool.tile([P, 2 * QCH], bf16, tag="PT",
                                          name="pt")
                        nc.scalar.activation(
                            pt[:, vs:], ssc[:, vs:], Exp, scale=0.125)
                        if ki >= 4 * qs:  # diagonal tile: mask both heads
                            blk = pt.rearrange(
                                "p (c w) -> p c w", c=2)[:, :, vs:vs + P]
                            nc.vector.tensor_tensor(
                                blk, blk,
                                tri_sb[:, None, :].to_broadcast(blk.shape),
                                mult)
                        nc.tensor.matmul(
                            za[:, vs:QCH], lhsT=V_sb[:, 0, ki, :],
                            rhs=pt[:, vs:QCH],
                            start=(ki == 0), stop=(ki == nk - 1))
                        nc.tensor.matmul(
                            zb[:, vs:QCH], lhsT=V_sb[:, 1, ki, :],
                            rhs=pt[:, QCH + vs:2 * QCH],
                            start=(ki == 0), stop=(ki == nk - 1))
                    if qs > 0:  # deferred: previous window's broadcast+apply
                        norm_stage2(s_ps, "S", [P, 2 * QCH], 0,
                                    q0 - QCH, QCH, staged[0])
                        norm_stage2(s_ps, "S", [P, 2 * QCH], 1,
                                    q0 - QCH, QCH, staged[1])
                    staged[0] = norm_stage1(za, QCH)
                    staged[1] = norm_stage1(zb, QCH)

            # ================= flash: head 2 (+ interleaved O-proj) =======
            with (
                tc.tile_pool(name="s2_ps", bufs=2, space="PSUM") as s2_ps,
                tc.tile_pool(name="z2_ps", bufs=2, space="PSUM") as z2_ps,
                tc.tile_pool(name="pt2_sb", bufs=3) as pt2_pool,
                tc.tile_pool(name="o_sb", bufs=3) as o_pool,
            ):
                def oproj(w2):
                    """Output projection for the 8 t-tiles of h2-window w2."""
                    for tt in range(w2 * (QS2 // P), (w2 + 1) * (QS2 // P)):
                        po = s2_ps.tile([P, max(QS2, DM)], f32, tag="S2",
                                        name="po")[:, :DM]
                        for h in range(HPC):
                            lhsT = Zn_sb[:, h, tt * P:(tt + 1) * P]
                            nc.tensor.matmul(po[:, 0:QCH], lhsT,
                                             rhs=wo_sb[:, h, 0:QCH],
                                             start=(h == 0),
                                             stop=(h == HPC - 1))
                            nc.tensor.matmul(po[:, QCH:DM], lhsT,
                                             rhs=wo_sb[:, h, QCH:DM],
                                             start=(h == 0),
                                             stop=(h == HPC - 1))
                        osb = o_pool.tile([P, DM], f32, tag="osb", name="osb")
                        nc.vector.tensor_copy(osb[:], po[:])
                        nc.sync.dma_start(out[tt * P:(tt + 1) * P, :], osb[:])

                staged2 = None
                for w2 in range(n_w2):
                    q0 = w2 * QS2
                    zacc = z2_ps.tile([DH + 1, QS2], f32, tag="zacc2",
                                      name="zacc")
                    nk = kpw2 * w2 + kpw2
                    for ki in range(nk):
                        vs = max(0, P * ki - q0)
                        ssc = s2_ps.tile([P, QS2], f32, tag="S2", name="ssc2")
                        for half in range(QS2 // QCH):
                            lo = max(vs, half * QCH)
                            hi = (half + 1) * QCH
                            if lo < hi:
                                nc.tensor.matmul(
                                    ssc[:, lo:hi],
                                    lhsT=KTs[:, ki * P:(ki + 1) * P],
                                    rhs=QTs[:, q0 + lo:q0 + hi],
                                    start=True, stop=True)
                        pt = pt2_pool.tile([P, QS2], bf16, tag="PT2",
                                           name="pt2")
                        nc.scalar.activation(
                            pt[:, vs:], ssc[:, vs:], Exp, scale=0.125)
                        if ki >= kpw2 * w2:
                            nc.vector.tensor_tensor(
                                pt[:, vs:vs + P], pt[:, vs:vs + P],
                                tri_sb[:], mult)
                        for half in range(QS2 // QCH):
                            lo = max(vs, half * QCH)
                            hi = (half + 1) * QCH
                            if lo < hi:
                                nc.tensor.matmul(
                                    zacc[:, lo:hi], lhsT=V_sb[:, 2, ki, :],
                                    rhs=pt[:, lo:hi],
                                    start=(ki == 0), stop=(ki == nk - 1))
                    if w2 == 0:  # finish the pair phase's last window
                        norm_stage2(s2_ps, "S2", [P, QS2], 0,
                                    SQ - QCH, QCH, staged[0])
                        norm_stage2(s2_ps, "S2", [P, QS2], 1,
                                    SQ - QCH, QCH, staged[1])
                    else:
                        norm_stage2(s2_ps, "S2", [P, QS2], 2,
                                    q0 - QS2, QS2, staged2)
                        oproj(w2 - 1)
                    staged2 = norm_stage1(zacc, QS2)
                norm_stage2(s2_ps, "S2", [P, QS2], 2,
                            SQ - QS2, QS2, staged2)
                oproj(n_w2 - 1)

    nc.compile()
    return nc


def _prep_inputs(inputs, seq_len, use_biases):
    x = np.asarray(inputs["normalized_resid_pre"], dtype=np.float32)
    WQ = np.asarray(inputs["W_Q"], dtype=np.float32)
    WK = np.asarray(inputs["W_K"], dtype=np.float32)
    WV = np.asarray(inputs["W_V"], dtype=np.float32)
    WO = np.asarray(inputs["W_O"], dtype=np.float32)

    tri = np.triu(np.ones((P, P), np.float32)).astype(_BF)  # keep j >= p
    idb = np.eye(P, dtype=np.float32).astype(_BF)
    onz = np.ones((1, DH), np.float32)

    in_maps = []
    for c in range(NCORES):
        b, g = divmod(c, GROUPS)
        hs = slice(g * HPC, (g + 1) * HPC)
        m = {
            "xT": np.ascontiguousarray(x[b, :seq_len].T).astype(_BF),
            "wq": np.ascontiguousarray(
                WQ[hs].transpose(1, 0, 2).reshape(DM, HPC * DH)).astype(_BF),
            "wk": np.ascontiguousarray(
                WK[hs].transpose(1, 0, 2).reshape(DM, HPC * DH)).astype(_BF),
            "wv": np.ascontiguousarray(
                WV[hs].transpose(1, 0, 2).reshape(DM, HPC * DH)).astype(_BF),
            "wo": np.ascontiguousarray(
                WO[hs].transpose(1, 0, 2).reshape(DH, HPC * DM)).astype(_BF),
            "trimask": tri,
            "ident_b": idb,
            "ones_z": onz,
        }
        if use_biases:
            bq = np.asarray(inputs["b_Q"], np.float32)[hs]
            bk = np.asarray(inputs["b_K"], np.float32)[hs]
            bv = np.asarray(inputs["b_V"], np.float32)[hs]
            # pair layout: [128, 3] = heads {0,1} stacked, cols q/k/v
            m["bqkv_p"] = np.stack(
                [np.concatenate([bq[0], bq[1]]),
                 np.concatenate([bk[0], bk[1]]),
                 np.concatenate([bv[0], bv[1]])], axis=1)
            m["bqkv_s"] = np.stack([bq[2], bk[2], bv[2]], axis=1)
        in_maps.append(m)
    return in_maps


TRACE = False          # test.py can flip this to get exec_time_ns
last_result = None     # BassKernelResults of the most recent run


def kernel(seq_len=S, **inputs):
    global last_result
    from concourse.bass_utils import run_bass_kernel_spmd

    use_biases = any(
        np.any(np.asarray(inputs[k]) != 0) for k in ("b_Q", "b_K", "b_V"))

    key = (seq_len, use_biases)
    if key not in _cache:
        _cache[key] = _build(seq_len, use_biases)
    nc = _cache[key]

    in_maps = _prep_inputs(inputs, seq_len, use_biases)
    res = run_bass_kernel_spmd(nc, in_maps, core_ids=list(range(NCORES)),
                               trace=TRACE)
    last_result = res

    b_O = np.asarray(inputs["b_O"], dtype=np.float32)
    out = np.zeros((B, seq_len, DM), np.float32)
    for c in range(NCORES):
        b = c // GROUPS
        out[b] += np.asarray(res.results[c]["out"], dtype=np.float32)
    out += b_O[None, None, :]
    return out


# revision 26
# speedup vs baseline: 1.0447x; 1.0447x over previous
"""Causal multi-head attention on 8 Trainium2 NeuronCores.

Problem: B=2, S=4096, D_MODEL=768, H=12, D_HEAD=64, fp32 I/O.

Sharding: (batch, head-group) -> core.  Cores 0-3 take batch 0, cores 4-7
take batch 1; each core computes 3 of the 12 heads for its batch and emits a
partial output [S, D_MODEL] (its heads' contribution to the W_O contraction).
The host sums the 4 partials per batch and adds b_O.

Per-core device program (matmul compute in bf16, fp32 PSUM accumulation):
  1. QT/KT[z, t] = W.T @ xT; heads 0,1 packed on partition halves (0-63 /
     64-127) so their scores matmuls run concurrently in different PE row
     groups; head 2 separate.  VT computed per head-pair/single, then
     PE-transposed to V[t, z] with a ones column appended (softmax row sums).
  2. Flash attention with scores in [k, q] layout so the exp output PT feeds
     the AV matmul directly; Z accumulates in PSUM [65 x W], row 64 = sum(P).
  3. Softmax normalization: row sums leave PSUM on DVE (32x32 block
     transposes + strided reciprocal), are broadcast across partitions by a
     rank-1 fp32r matmul, and applied with one DVE multiply.  The broadcast
     matmul for window w is emitted after window w+1's score matmuls so its
     DVE-side inputs are always ready and the PE never stalls (stalling >3.4us
     re-throttles the PE clock from 2.4 to 1.2 GHz).
  4. Output projection accumulates all 3 heads into PSUM [t, 768]; emitted
     interleaved with the head-2 windows to keep the PE dense.
"""

import numpy as np
import ml_dtypes

B, S, DM, H, DH = 2, 4096, 768, 12, 64
NCORES = 8
GROUPS = 4                  # head-groups per batch
HPC = H // GROUPS           # heads per core = 3
P = 128
QCH = 512                   # psum bank width (fp32)

_BF = ml_dtypes.bfloat16

_cache = {}


def _build(seq_len, use_biases):
    import concourse.bacc as bacc
    import concourse.mybir as mybir
    import concourse.tile as tile

    f32 = mybir.dt.float32
    f32r = mybir.dt.float32r
    bf16 = mybir.dt.bfloat16
    Exp = mybir.ActivationFunctionType.Exp
    mult = mybir.AluOpType.mult

    SQ = seq_len
    n_kt = SQ // P               # k tiles
    n_tt = SQ // P               # output row tiles
    n_ch = SQ // QCH             # 512-wide chunks
    DSL = DM // P                # contraction slices for the projections
    QS2 = min(2 * QCH, SQ)       # head-2 flash window
    n_w2 = SQ // QS2
    kpw2 = QS2 // P

    nc = bacc.Bacc(None, target_bir_lowering=False)

    xT = nc.declare_dram_parameter("xT", [DM, SQ], bf16, isOutput=False)
    wq = nc.declare_dram_parameter("wq", [DM, HPC * DH], bf16, isOutput=False)
    wk = nc.declare_dram_parameter("wk", [DM, HPC * DH], bf16, isOutput=False)
    wv = nc.declare_dram_parameter("wv", [DM, HPC * DH], bf16, isOutput=False)
    wo = nc.declare_dram_parameter("wo", [DH, HPC * DM], bf16, isOutput=False)
    trimask = nc.declare_dram_parameter("trimask", [P, P], bf16, isOutput=False)
    ident_b = nc.declare_dram_parameter("ident_b", [P, P], bf16, isOutput=False)
    ones_z = nc.declare_dram_parameter("ones_z", [1, DH], f32r, isOutput=False)
    if use_biases:
        bqkv_p = nc.declare_dram_parameter("bqkv_p", [P, 3], f32, isOutput=False)
        bqkv_s = nc.declare_dram_parameter("bqkv_s", [DH, 3], f32, isOutput=False)
    out = nc.declare_dram_parameter("out", [SQ, DM], f32, isOutput=True)

    with tile.TileContext(nc) as tc:
        with (
            tc.tile_pool(name="singles", bufs=1) as singles,
            tc.tile_pool(name="persist", bufs=1) as persist,
            tc.tile_pool(name="nrm_t", bufs=2) as nrm_t,
            tc.tile_pool(name="nrm_k", bufs=4) as nrm_k,
        ):
            # ---- constants / weights ----
            w_sb = {}
            for name, drm in (("q", wq), ("k", wk), ("v", wv)):
                t = singles.tile([P, DSL, HPC * DH], bf16, tag=f"w{name}")
                nc.sync.dma_start(t[:], drm.rearrange("(o p) c -> p o c", p=P))
                w_sb[name] = t
            wo_sb = singles.tile([DH, HPC, DM], bf16)
            nc.sync.dma_start(wo_sb[:], wo.rearrange("z (h d) -> z h d", h=HPC))
            tri_sb = singles.tile([P, P], bf16)
            nc.sync.dma_start(tri_sb[:], trimask[:])
            idb_sb = singles.tile([P, P], bf16)
            nc.sync.dma_start(idb_sb[:], ident_b[:])
            ones_sb = singles.tile([1, DH], f32r)
            nc.sync.dma_start(ones_sb[:], ones_z[:])
            bias_p = bias_s = None
            if use_biases:
                bias_p = singles.tile([P, 3], f32, tag="bp")
                nc.sync.dma_start(bias_p[:], bqkv_p[:])
                bias_s = singles.tile([DH, 3], f32, tag="bs")
                nc.sync.dma_start(bias_s[:], bqkv_s[:])

            # ---- persistent activations ----
            QT2 = persist.tile([P, SQ], bf16, tag="QT2")   # heads 0,1 stacked
            KT2 = persist.tile([P, SQ], bf16, tag="KT2")
            QTs = persist.tile([DH, SQ], bf16, tag="QTs")  # head 2
            KTs = persist.tile([DH, SQ], bf16, tag="KTs")
            V_sb = persist.tile([P, HPC, n_kt, DH + 1], bf16, tag="V")
            Zn_sb = persist.tile([DH, HPC, SQ], bf16, tag="Zn")

            # ================= QKV projections =================
            with (
                tc.tile_pool(name="xT_pool", bufs=1) as xT_pool,
                tc.tile_pool(name="qkv_ps", bufs=3, space="PSUM") as qkv_ps,
                tc.tile_pool(name="vt_ps", bufs=3, space="PSUM") as vt_ps,
                tc.tile_pool(name="vt_sb", bufs=1) as vt_pool,
            ):
                xT_sb = xT_pool.tile([P, DSL, SQ], bf16)
                for c in range(n_ch):
                    for o in range(DSL):
                        nc.sync.dma_start(
                            xT_sb[:, o, c * QCH:(c + 1) * QCH],
                            xT[o * P:(o + 1) * P, c * QCH:(c + 1) * QCH])

                def proj(tname, w_cols, dst, bias, c):
                    m = w_cols.stop - w_cols.start
                    ps = qkv_ps.tile([P, QCH], f32, tag="proj",
                                     name="proj_ps")[:m]
                    for o in range(DSL):
                        nc.tensor.matmul(
                            ps[:],
                            lhsT=w_sb[tname][:, o, w_cols],
                            rhs=xT_sb[:, o, c * QCH:(c + 1) * QCH],
                            start=(o == 0), stop=(o == DSL - 1),
                        )
                    if bias is not None:
                        nc.scalar.add(dst, ps[:], bias)
                    else:
                        nc.vector.tensor_copy(dst, ps[:])

                for tname, d2, ds, bi in (("q", QT2, QTs, 0),
                                          ("k", KT2, KTs, 1)):
                    for c in range(n_ch):
                        proj(tname, slice(0, P),
                             d2[:, c * QCH:(c + 1) * QCH],
                             bias_p[:, bi:bi + 1] if use_biases else None, c)
                    for c in range(n_ch):
                        proj(tname, slice(P, P + DH),
                             ds[:, c * QCH:(c + 1) * QCH],
                             bias_s[:, bi:bi + 1] if use_biases else None, c)
                # V: pair pass (M=128) + single pass, then batched transposes
                vt2 = vt_pool.tile([P, SQ], bf16, tag="vt2")
                for c in range(n_ch):
                    proj("v", slice(0, P), vt2[:, c * QCH:(c + 1) * QCH],
                         bias_p[:, 2:3] if use_biases else None, c)
                vts = vt_pool.tile([DH, SQ], bf16, tag="vts")
                for c in range(n_ch):
                    proj("v", slice(P, P + DH),
                         vts[:, c * QCH:(c + 1) * QCH],
                         bias_s[:, 2:3] if use_biases else None, c)
                for kt in range(n_kt):
                    for h, src, idsl in (
                            (0, vt2[0:DH, kt * P:(kt + 1) * P],
                             idb_sb[:DH, :DH]),
                            (1, vt2[DH:P, kt * P:(kt + 1) * P],
                             idb_sb[DH:P, DH:P]),
                            (2, vts[:, kt * P:(kt + 1) * P],
                             idb_sb[:DH, :DH])):
                        vp = vt_ps.tile([P, DH], bf16, tag="vtp", name="vp")
                        nc.tensor.transpose(vp[:], src, idsl)
                        nc.vector.tensor_copy(V_sb[:, h, kt, 0:DH], vp[:])
                nc.vector.memset(V_sb[:, :, :, DH:DH + 1], 1.0)

            # ---- normalization helpers (DVE stage + deferred PE stage) ----
            def norm_stage1(zacc, width):
                """All-DVE: extract row sums, reciprocal, repack; copy Z."""
                rr_pieces = []
                for j in range(width // QCH):
                    sl = slice(j * QCH, (j + 1) * QCH)
                    r32 = nrm_t.tile([32, QCH], f32, tag="r32", name="r32")
                    nc.vector.tensor_copy(r32[0:1, :], zacc[DH:DH + 1, sl])
                    rT = nrm_t.tile([32, QCH], f32, tag="rT", name="rT")
                    nc.vector.transpose(rT[:], r32[:])
                    rrT = nrm_t.tile([32, QCH], f32, tag="rrT", name="rrT")
                    nc.vector.reciprocal(
                        rrT.rearrange("p (j c) -> p j c", c=32)[:, :, 0],
                        rT.rearrange("p (j c) -> p j c", c=32)[:, :, 0])
                    rr32 = nrm_t.tile([32, QCH], f32, tag="rr32", name="rr32")
                    nc.vector.transpose(rr32[:], rrT[:])
                    rr_sb = nrm_k.tile([1, QCH], f32r, tag="rr", name="rr_sb")
                    nc.vector.tensor_copy(rr_sb[:], rr32[0:1, :])
                    rr_pieces.append(rr_sb)
                zsb = nrm_k.tile([DH, width], bf16, tag="zsb", name="zsb")
                nc.vector.tensor_copy(zsb[:], zacc[0:DH, :])
                return rr_pieces, zsb

            def norm_stage2(ps_pool, ps_tag, ps_shape, h, q0, width, staged):
                """PE rank-1 broadcast of 1/r, then one DVE multiply."""
                rr_pieces, zsb = staged
                rrb = ps_pool.tile(ps_shape, f32, tag=ps_tag, name="rrb")
                for half in range(width // QCH):
                    sl = slice(half * QCH, (half + 1) * QCH)
                    nc.tensor.matmul(rrb[:DH, sl], lhsT=ones_sb[:],
                                     rhs=rr_pieces[half][:],
                                     start=True, stop=True)
                nc.vector.tensor_tensor(
                    Zn_sb[:, h, q0:q0 + width], zsb[:], rrb[:DH, :width], mult)

            # ===== flash: all heads interleaved, one 512-wide window loop ====
            with (
                tc.tile_pool(name="s_ps", bufs=2, space="PSUM") as s_ps,
                tc.tile_pool(name="z_ps", bufs=4, space="PSUM") as z_ps,
                tc.tile_pool(name="pt_sb", bufs=3) as pt_pool,
                tc.tile_pool(name="o_sb", bufs=3) as o_pool,
            ):
                SW = max(2 * QCH, DM)   # shared psum slot width (tag "S")

                def oproj(w):
                    """Output projection for the 4 t-tiles of window w."""
                    for tt in range(w * (QCH // P), (w + 1) * (QCH // P)):
                        po = s_ps.tile([P, SW], f32, tag="S",
                                       name="po")[:, :DM]
                        for h in range(HPC):
                            lhsT = Zn_sb[:, h, tt * P:(tt + 1) * P]
                            nc.tensor.matmul(po[:, 0:QCH], lhsT,
                                             rhs=wo_sb[:, h, 0:QCH],
                                             start=(h == 0),
                                             stop=(h == HPC - 1))
                            nc.tensor.matmul(po[:, QCH:DM], lhsT,
                                             rhs=wo_sb[:, h, QCH:DM],
                                             start=(h == 0),
                                             stop=(h == HPC - 1))
                        osb = o_pool.tile([P, DM], f32, tag="osb", name="osb")
                        nc.vector.tensor_copy(osb[:], po[:])
                        nc.sync.dma_start(out[tt * P:(tt + 1) * P, :], osb[:])

                staged = {}
                for qs in range(n_ch):
                    q0 = qs * QCH
                    za = z_ps.tile([DH + 1, QCH], f32, tag="zacc", name="za")
                    zb = z_ps.tile([DH + 1, QCH], f32, tag="zacc", name="zb")
                    zc = z_ps.tile([DH + 1, QCH], f32, tag="zacc", name="zc")
                    nk = 4 * qs + 4
                    # --- heads 0,1: concurrent scores in two PE row groups ---
                    for ki in range(nk):
                        vs = max(0, P * ki - q0)
                        ssc = s_ps.tile([P, SW], f32, tag="S", name="ssc")
                        nc.tensor.matmul(
                            ssc[:, vs:QCH],
                            lhsT=KT2[0:DH, ki * P:(ki + 1) * P],
                            rhs=QT2[0:DH, q0 + vs:q0 + QCH],
                            start=True, stop=True)
                        nc.tensor.matmul(
                            ssc[:, QCH + vs:2 * QCH],
                            lhsT=KT2[DH:P, ki * P:(ki + 1) * P],
                            rhs=QT2[DH:P, q0 + vs:q0 + QCH],
                            start=True, stop=True)
                        pt = pt_pool.tile([P, 2 * QCH], bf16, tag="PT",
                                          name="pt")
                        nc.scalar.activation(
                            pt[:, vs:], ssc[:, vs:2 * QCH], Exp, scale=0.125)
                        if ki >= 4 * qs:  # diagonal tile: mask both heads
                            blk = pt.rearrange(
                                "p (c w) -> p c w", c=2)[:, :, vs:vs + P]
                            nc.vector.tensor_tensor(
                                blk, blk,
                                tri_sb[:, None, :].to_broadcast(blk.shape),
                                mult)
                        nc.tensor.matmul(
                            za[:, vs:QCH], lhsT=V_sb[:, 0, ki, :],
                            rhs=pt[:, vs:QCH],
                            start=(ki == 0), stop=(ki == nk - 1))
                        nc.tensor.matmul(
                            zb[:, vs:QCH], lhsT=V_sb[:, 1, ki, :],
                            rhs=pt[:, QCH + vs:2 * QCH],
                            start=(ki == 0), stop=(ki == nk - 1))
                    # --- head 2 ---
                    for ki in range(nk):
                        vs = max(0, P * ki - q0)
                        ssc = s_ps.tile([P, SW], f32, tag="S",
                                        name="ssc2")[:, :QCH]
                        nc.tensor.matmul(
                            ssc[:, vs:QCH],
                            lhsT=KTs[:, ki * P:(ki + 1) * P],
                            rhs=QTs[:, q0 + vs:q0 + QCH],
                            start=True, stop=True)
                        pt = pt_pool.tile([P, QCH], bf16, tag="PT2",
                                          name="pt2")
                        nc.scalar.activation(
                            pt[:, vs:], ssc[:, vs:], Exp, scale=0.125)
                        if ki >= 4 * qs:
                            nc.vector.tensor_tensor(
                                pt[:, vs:vs + P], pt[:, vs:vs + P],
                                tri_sb[:], mult)
                        nc.tensor.matmul(
                            zc[:, vs:QCH], lhsT=V_sb[:, 2, ki, :],
                            rhs=pt[:, vs:QCH],
                            start=(ki == 0), stop=(ki == nk - 1))
                    # --- deferred normalization + O-proj of previous window ---
                    if qs > 0:
                        for h in range(HPC):
                            norm_stage2(s_ps, "S", [P, SW], h,
                                        q0 - QCH, QCH, staged[h])
                        oproj(qs - 1)
                    for h, z in ((0, za), (1, zb), (2, zc)):
                        staged[h] = norm_stage1(z, QCH)
                for h in range(HPC):
                    norm_stage2(s_ps, "S", [P, SW], h,
                                SQ - QCH, QCH, staged[h])
                oproj(n_ch - 1)

    nc.compile()
    return nc


def _prep_inputs(inputs, seq_len, use_biases):
    x = np.asarray(inputs["normalized_resid_pre"], dtype=np.float32)
    WQ = np.asarray(inputs["W_Q"], dtype=np.float32)
    WK = np.asarray(inputs["W_K"], dtype=np.float32)
    WV = np.asarray(inputs["W_V"], dtype=np.float32)
    WO = np.asarray(inputs["W_O"], dtype=np.float32)

    tri = np.triu(np.ones((P, P), np.float32)).astype(_BF)  # keep j >= p
    idb = np.eye(P, dtype=np.float32).astype(_BF)
    onz = np.ones((1, DH), np.float32)

    in_maps = []
    for c in range(NCORES):
        b, g = divmod(c, GROUPS)
        hs = slice(g * HPC, (g + 1) * HPC)
        m = {
            "xT": np.ascontiguousarray(x[b, :seq_len].T).astype(_BF),
            "wq": np.ascontiguousarray(
                WQ[hs].transpose(1, 0, 2).reshape(DM, HPC * DH)).astype(_BF),
            "wk": np.ascontiguousarray(
                WK[hs].transpose(1, 0, 2).reshape(DM, HPC * DH)).astype(_BF),
            "wv": np.ascontiguousarray(
                WV[hs].transpose(1, 0, 2).reshape(DM, HPC * DH)).astype(_BF),
            "wo": np.ascontiguousarray(
                WO[hs].transpose(1, 0, 2).reshape(DH, HPC * DM)).astype(_BF),
            "trimask": tri,
            "ident_b": idb,
            "ones_z": onz,
        }
        if use_biases:
            bq = np.asarray(inputs["b_Q"], np.float32)[hs]
            bk = np.asarray(inputs["b_K"], np.float32)[hs]
            bv = np.asarray(inputs["b_V"], np.float32)[hs]
            # pair layout: [128, 3] = heads {0,1} stacked, cols q/k/v
            m["bqkv_p"] = np.stack(
                [np.concatenate([bq[0], bq[1]]),
                 np.concatenate([bk[0], bk[1]]),
                 np.concatenate([bv[0], bv[1]])], axis=1)
            m["bqkv_s"] = np.stack([bq[2], bk[2], bv[2]], axis=1)
        in_maps.append(m)
    return in_maps


TRACE = False          # test.py can flip this to get exec_time_ns
last_result = None     # BassKernelResults of the most recent run


def kernel(seq_len=S, **inputs):
    global last_result
    from concourse.bass_utils import run_bass_kernel_spmd

    use_biases = any(
        np.any(np.asarray(inputs[k]) != 0) for k in ("b_Q", "b_K", "b_V"))

    key = (seq_len, use_biases)
    if key not in _cache:
        _cache[key] = _build(seq_len, use_biases)
    nc = _cache[key]

    in_maps = _prep_inputs(inputs, seq_len, use_biases)
    res = run_bass_kernel_spmd(nc, in_maps, core_ids=list(range(NCORES)),
                               trace=TRACE)
    last_result = res

    b_O = np.asarray(inputs["b_O"], dtype=np.float32)
    out = np.zeros((B, seq_len, DM), np.float32)
    for c in range(NCORES):
        b = c // GROUPS
        out[b] += np.asarray(res.results[c]["out"], dtype=np.float32)
    out += b_O[None, None, :]
    return out
